# revision 1
# baseline (speedup 1.0000x reference)
"""Trainium2 Bass kernel for CRF logZ (nn_CRFModel).

Math: probability-space forward recurrence with a constant per-step rescale
folded into the transitions (expAs = exp(A - log64)); the state
p~ = exp(alpha - t*log64) stays in ~[1e-5, 1e-1] so no per-step
normalization is needed.  logZ = log(expAs[:,EOS]^T p~_T) + 129*log64.

Per core (data-parallel, 32 sentences each):
  1. xbar dma_gather(transpose=True) pulls the 4096 needed E rows (fp16)
     from two half-vocab tables (int16 index limit) directly in
     D-on-partitions layout: out[p, c, w] = E[word_w, 128c+p].
  2. copy_predicated merges the two gathers (hi-vocab words overwrite).
  3. GEMM emis[tag, w] = ThetaB @ Erows^T in fp16, N=512 per matmul.
  4. exp on ScalarE -> expE.
  5. 128-step recurrence split into two 16-sentence chains, phase-
     interleaved so PE/DVE semaphore latency of one chain hides under the
     other's work: q = expAs^T p (PE, fp16), p' = q * expE_t (DVE).
Masking: expAs[:, BOS]=0, expAs[EOS, :]=0, and the final contraction
column has EOS entry 0 - exactly equivalent to the reference's NEG masks.
"""

import sys

for _p in ("/opt/trn_rl_repo", "/root/.axon_site/_ro/trn_rl_repo"):
    if _p not in sys.path:
        sys.path.insert(0, _p)

import math

import numpy as np

import concourse.bass as bass
import concourse.mybir as mybir
import concourse.tile as tile
from concourse import bacc
from concourse.bass_utils import run_bass_kernel_spmd
from concourse.tile import add_dep_helper


K = 64
V = 50257
D = 512
BT = 256
T = 128
BOS = 62
EOS = 63
N_CORES = 8
B_PER_CORE = BT // N_CORES          # 32 sentences per core
HB = B_PER_CORE // 2                # 16 sentences per chain
W_PER_CORE = B_PER_CORE * T         # 4096 gathered words per core
VSPLIT = 32768                      # int16 index limit
NW_G = 512                          # max words per gather instruction
# words per gather group (tried [128,384]+[512]*7 to shrink the head: the
# first mul starts 11us sooner but the extra group boundaries stretch the
# PE-saturated recurrence by the same amount - uniform is best measured).
GROUPS = [512] * 8
assert sum(GROUPS) == W_PER_CORE
N_G = len(GROUPS)
LOG64 = math.log(64.0)

F32 = mybir.dt.float32
F16 = mybir.dt.float16
I16 = mybir.dt.int16
U8 = mybir.dt.uint8

_CACHE = {}


def _build():
    nc = bacc.Bacc("TRN2", target_bir_lowering=False, debug=False,
                   num_devices=N_CORES)

    S = W_PER_CORE // 16  # 256 idx slots per partition-row
    idx_d = nc.dram_tensor("idx2", [128, 2 * S], I16, kind="ExternalInput").ap()
    msk_d = nc.dram_tensor("maskhi", [128, 4 * W_PER_CORE], U8,
                           kind="ExternalInput").ap()
    wa_d = nc.dram_tensor("WA", [K, K], F32, kind="ExternalInput").ap()
    amask_d = nc.dram_tensor("amask", [K, K], F32, kind="ExternalInput").ap()
    thT_d = nc.dram_tensor("ThetaBT", [4, 128, K], F16,
                           kind="ExternalInput").ap()
    p0_d = nc.dram_tensor("p0", [K, HB], F16, kind="ExternalInput").ap()
    elo_d = nc.dram_tensor("Elo", [VSPLIT, D], F16, kind="ExternalInput").ap()
    ehi_d = nc.dram_tensor("Ehi", [V - VSPLIT, D], F16,
                           kind="ExternalInput").ap()
    out_d = nc.dram_tensor("out", [1, B_PER_CORE], F32,
                           kind="ExternalOutput").ap()

    with tile.TileContext(nc) as tc:
        with (
            tc.tile_pool(name="const", bufs=1) as cpool,
            tc.tile_pool(name="gat", bufs=8) as gpool,
            tc.tile_pool(name="pst", bufs=3) as ppool,
            tc.tile_pool(name="psum_em", bufs=2, space="PSUM") as ps_em,
            tc.tile_pool(name="psum_qa", bufs=3, space="PSUM") as ps_qa,
            tc.tile_pool(name="psum_qb", bufs=3, space="PSUM") as ps_qb,
        ):
            # ---- constants ------------------------------------------------
            # one combined idx DMA first: the gathers gate on nothing else
            idx2 = cpool.tile([128, 2 * S], I16, tag="idx2")
            nc.gpsimd.dma_start(idx2[:], idx_d[:])
            ilo = idx2[:, 0:S]
            ihi = idx2[:, S:2 * S]

            msks = []
            moff = 0
            for g, nw in enumerate(GROUPS):
                m_g = cpool.tile([128, 4 * nw], U8, tag=f"msk{g}")
                nc.sync.dma_start(m_g[:], msk_d[:, moff:moff + 4 * nw])
                msks.append(m_g)
                moff += 4 * nw

            wa_sb = cpool.tile([K, K], F32, tag="wa")
            nc.sync.dma_start(wa_sb[:], wa_d[:])
            amask = cpool.tile([K, K], F32, tag="amask")
            nc.sync.dma_start(amask[:], amask_d[:])

            # expAs = exp(WA - log64) * mask (mask: col BOS = 0, row EOS = 0)
            nlog64 = cpool.tile([K, 1], F32, tag="nlog64")
            nc.vector.memset(nlog64[:], -LOG64)
            expas = cpool.tile([K, K], F32, tag="expas")
            nc.scalar.activation(expas[:], wa_sb[:],
                                 mybir.ActivationFunctionType.Exp,
                                 bias=nlog64[:], scale=1.0)
            expas_bf = cpool.tile([K, K], F16, tag="expas_bf")
            nc.vector.tensor_mul(expas_bf[:], expas[:], amask[:])

            # ThetaB^T fp16 chunks [128, 64] (host pre-transposed)
            thT = []
            for c in range(4):
                t_bf = cpool.tile([128, K], F16, tag=f"thT{c}")
                nc.sync.dma_start(t_bf[:], thT_d[c])
                thT.append(t_bf)

            # initial state p0 = one-hot(BOS), two half-batch chains
            pA = ppool.tile([K, HB], F16, tag="pA")
            nc.sync.dma_start(pA[:], p0_d[:])
            pB = ppool.tile([K, HB], F16, tag="pB")
            nc.sync.dma_start(pB[:], p0_d[:])

            # ---- pipeline over 8 groups of 512 words (16 steps each) ------
            # Order-only anchors so the scheduler interleaves each group's
            # emission work into the previous group's recurrence instead of
            # running the whole emission phase first (PE/DVE are FIFO).
            rec_mm = []   # recurrence matmul instructions of previous group
            rec_mul = []  # recurrence multiply instructions of previous group
            woff = 0
            for g, nw in enumerate(GROUPS):
                sl = slice(woff // 16, (woff + nw) // 16)
                glo = gpool.tile([128, 4 * nw], F16, tag="glo")
                nc.gpsimd.dma_gather(
                    glo[:].rearrange("p (c w) -> p c w", c=4),
                    elo_d[:], ilo[:, sl], nw, nw, D, transpose=True)
                ghi = gpool.tile([128, 4 * nw], F16, tag="ghi")
                nc.gpsimd.dma_gather(
                    ghi[:].rearrange("p (c w) -> p c w", c=4),
                    ehi_d[:], ihi[:, sl], nw, nw, D, transpose=True)
                mrg = nc.vector.copy_predicated(glo[:], msks[g][:], ghi[:])
                if rec_mul:
                    add_dep_helper(mrg.ins, rec_mul[len(rec_mul) // 4].ins,
                                   reason="interleave merge into prev recurrence")

                em_ps = ps_em.tile([K, nw], F32, tag="em")
                for c in range(4):
                    mm = nc.tensor.matmul(em_ps[:], lhsT=thT[c][:],
                                          rhs=glo[:, c * nw:(c + 1) * nw],
                                          start=(c == 0), stop=(c == 3))
                    if rec_mm and c == 0:
                        add_dep_helper(mm.ins, rec_mm[(len(rec_mm) * 5) // 8].ins,
                                       reason="interleave gemm into prev recurrence")
                expe = cpool.tile([K, nw], F32, tag=f"expe{g}")
                nc.scalar.activation(expe[:], em_ps[:],
                                     mybir.ActivationFunctionType.Exp)

                rec_mm, rec_mul = [], []
                for tt in range(nw // B_PER_CORE):
                    w0 = tt * B_PER_CORE
                    qa = ps_qa.tile([K, HB], F32, tag="qa")
                    rec_mm.append(
                        nc.tensor.matmul(qa[:], lhsT=expas_bf[:], rhs=pA[:],
                                         start=True, stop=True))
                    qb = ps_qb.tile([K, HB], F32, tag="qb")
                    rec_mm.append(
                        nc.tensor.matmul(qb[:], lhsT=expas_bf[:], rhs=pB[:],
                                         start=True, stop=True))
                    pA = ppool.tile([K, HB], F16, tag="pA")
                    rec_mul.append(
                        nc.vector.tensor_mul(pA[:], qa[:],
                                             expe[:, w0:w0 + HB]))
                    pB = ppool.tile([K, HB], F16, tag="pB")
                    rec_mul.append(
                        nc.vector.tensor_mul(pB[:], qb[:],
                                             expe[:, w0 + HB:w0 + B_PER_CORE]))
                woff += nw

            # ---- finale ---------------------------------------------------
            z = ps_em.tile([1, B_PER_CORE], F32, tag="em")
            nc.tensor.matmul(z[:, 0:HB], lhsT=expas_bf[:, EOS:EOS + 1],
                             rhs=pA[:], start=True, stop=True)
            nc.tensor.matmul(z[:, HB:B_PER_CORE],
                             lhsT=expas_bf[:, EOS:EOS + 1],
                             rhs=pB[:], start=True, stop=True)
            lnz = cpool.tile([1, B_PER_CORE], F32, tag="lnz")
            nc.scalar.activation(lnz[:], z[:], mybir.ActivationFunctionType.Ln)
            res = cpool.tile([1, B_PER_CORE], F32, tag="res")
            nc.vector.tensor_scalar_add(res[:], lnz[:], float((T + 1) * LOG64))
            nc.sync.dma_start(out_d[:], res[:])

    nc.compile()
    return nc


def _get_nc():
    if "nc" not in _CACHE:
        _CACHE["nc"] = _build()
    return _CACHE["nc"]


def _wrap16(w):
    """idx j -> partition j%16, slot j//16; replicated to all 8 Q7 cores."""
    a = np.asarray(w, np.int16).reshape(-1, 16).T  # [16, S]
    return np.tile(a, (8, 1))                      # [128, S]


def _make_in_maps(words, WA, ThetaB, E):
    words = np.asarray(words)
    WA = np.ascontiguousarray(np.asarray(WA, np.float32))
    ThetaB = np.asarray(ThetaB, np.float32)
    E = np.asarray(E, np.float32)
    Elo = np.ascontiguousarray(E[:VSPLIT].astype(np.float16))
    Ehi = np.ascontiguousarray(E[VSPLIT:].astype(np.float16))
    # ThetaB^T [512, 64] -> [4, 128, 64] fp16 chunks
    ThT = np.ascontiguousarray(
        ThetaB.T.reshape(4, 128, K).astype(np.float16))
    amask = np.ones((K, K), np.float32)
    amask[:, BOS] = 0.0
    amask[EOS, :] = 0.0
    p0 = np.zeros((K, HB), np.float16)
    p0[BOS, :] = 1.0

    in_maps = []
    for c in range(N_CORES):
        wb = words[c * B_PER_CORE:(c + 1) * B_PER_CORE].astype(np.int64)
        wf = wb.T.reshape(-1)                    # t-major flat: j = t*32 + b
        is_hi = wf >= VSPLIT
        wlo = np.where(is_hi, 0, wf).astype(np.int16)
        whi = np.where(is_hi, wf - VSPLIT, 0).astype(np.int16)
        parts, off = [], 0
        for nw in GROUPS:
            parts.append(np.tile(is_hi[off:off + nw], 4))
            off += nw
        m = np.concatenate(parts)
        mask = np.repeat(m.astype(np.uint8)[None, :], 128, axis=0)
        in_maps.append({
            "idx2": np.ascontiguousarray(
                np.concatenate([_wrap16(wlo), _wrap16(whi)], axis=1)),
            "maskhi": np.ascontiguousarray(mask),
            "WA": WA, "amask": amask, "ThetaBT": ThT, "p0": p0,
            "Elo": Elo, "Ehi": Ehi,
        })
    return in_maps


def kernel(words, WA, ThetaB, E):
    nc = _get_nc()
    in_maps = _make_in_maps(words, WA, ThetaB, E)
    res = run_bass_kernel_spmd(nc, in_maps, list(range(N_CORES)))
    return np.concatenate(
        [res.results[c]["out"][0] for c in range(N_CORES)]).astype(np.float32)

